# revision 2
# baseline (speedup 1.0000x reference)
"""Trainium2 Bass kernel v2: RK4 neural-ODE solver, u-space formulation.

Reference: f(h) = tanh(tanh(h@W1+b1)@W2+b2)@W3 + b3, RK4 over 199 steps,
outputs all states [B, T, H].

Key reformulation (vs the v1 kernel)
------------------------------------
Track u := h@W1 - C(t)*(b3@W1) (layer-1 preactivation, drift-corrected)
instead of feeding h through W1 every eval.  Since layers 1 and 3 are
linear, with M := W3@W1 [100,100]:

  z1p_e   = u + c_e * M.T@z2_{e-1} + bias_e(t)      (c_1=c_2=dt/2, c_3=dt)
  u_next  = u + (dt/6) * M.T@(z2_0 + 2 z2_1 + 2 z2_2 + z2_3)
  h_i     = u_i @ pinv(W1) + C_i*b3                 (output reconstruction)

All b1/b3 contributions fold into per-step host-precomputed bias tables
(exact per-step dt); the c_e / dt/6 scales fold into pre-scaled stationary
weights (mean-dt; per-step deviation is ~1e-6 relative, far below tol).
W1 is full-rank 64x100 (cond ~11), so the pinv reconstruction is exact in
real arithmetic and adds only ~5e-4 relative noise in f32r - and output
errors do NOT feed back into the state.

Consequences:
  * Critical chain per eval is exactly act -> mm -> act -> mm (4 hops,
    2 engines); DVE has NO ops on the chain.
  * u lives in a persistent PSUM bank updated purely by PE accumulation
    (start=False matmuls with stationaries (dt/6)M, (dt/3)M) - exact f32.
  * The step boundary collapses to: act(z2_3) -> mm((dt/6)M @ z2_3 into u)
    -> act(z1_0 next step, reading the u bank directly with bias).
  * Premix banks P_e = I@u_mm (off-chain) + c_e M@z2_{e-1} (on chain)
    feed the e>=1 layer-1 activations; u_mm is a once-per-step DVE f32r
    copy of the u bank - which ALSO feeds the output matmul pinv(W1)@u_mm,
    so there is no h-state recursion at all outside PSUM.

Layout: batch 4096 -> 8 cores x 512; per core NSTREAM independent
streams pipeline against each other (per-half-eval interleave keeps the
in-order Activation queue dense).  Output written time-major, staged
OUT_GROUP steps per DMA, host-transposed.
"""

import os
import sys

import numpy as np

for _p in ("/opt/trn_rl_repo", "/root/.axon_site/_ro/trn_rl_repo"):
    if os.path.isdir(_p) and _p not in sys.path:
        sys.path.insert(0, _p)

os.environ.setdefault("NEURON_SCRATCHPAD_PAGE_SIZE", "4096")

import concourse.bass as bass
import concourse.mybir as mybir
import concourse.tile as tile
from concourse.bass_utils import run_bass_kernel_spmd

F32 = mybir.dt.float32
F32R = mybir.dt.float32r
AF = mybir.ActivationFunctionType
OP = mybir.AluOpType

N_CORES = 8
H = 64
HT = 100
NSTREAM = 2

OUT_GROUP = 8
Z1_BUFS = 2
Z2_BUFS = 3
# PSUM is 8 banks of 2KB/partition; tiles round up to a full bank. Per
# stream: u, P (premix), zp (layer-2) bufs=1 (every WAR edge they add is
# already implied by the act chain order) = 6 banks; hout (output recon)
# is a shared tag with bufs=2 = 2 banks.  Total 8.
P_BUFS = 1
ZP_BUFS = 1
HOUT_BUFS = 2
MM_FAST = True


def _legalize_waits(nc: bass.Bass, max_waits: int = 1) -> int:
    """Walrus encodes at most ONE sync-wait per instruction; hoist extras
    onto injected same-engine NoOps (engine streams are in-order)."""
    self_sem_prefix = {
        mybir.EngineType.Activation: "Activation_",
        mybir.EngineType.PE: "PE_",
        mybir.EngineType.DVE: "DVE_",
        mybir.EngineType.Pool: "Pool_",
    }
    n_new = 0
    for fn in nc.m.functions:
        for bb in fn.blocks:
            new_list = []
            changed = False
            for ins in bb.instructions:
                si = ins.sync_info
                waits = list(si.on_wait) if si and si.on_wait else []
                pref = self_sem_prefix.get(ins.engine)
                if pref is not None and any(
                    (w.ant_name or "").startswith(pref) for w in waits
                ):
                    waits = [w for w in waits
                             if not (w.ant_name or "").startswith(pref)]
                    ins.sync_info = mybir.SyncInfo(
                        on_wait=list(waits),
                        on_update=list(si.on_update) if si.on_update else [],
                    )
                    changed = True
                    si = ins.sync_info
                if len(waits) > max_waits:
                    keep = waits[-max_waits:]
                    for w in waits[:-max_waits]:
                        nop = mybir.InstNoOp(name=f"I-waitsplit-{n_new}")
                        n_new += 1
                        nop.engine = ins.engine
                        nop.sync_info = mybir.SyncInfo(on_wait=[w], on_update=[])
                        new_list.append(nop)
                    ins.sync_info = mybir.SyncInfo(
                        on_wait=keep,
                        on_update=list(si.on_update) if si.on_update else [],
                    )
                    changed = True
                new_list.append(ins)
            if changed:
                bb.instructions = new_list
    return n_new


def make_feeds(h0, t, W1, b1, W2, b2, W3, b3, b_local, core):
    """Host-side precompute: per-core input map for build_program's tensors."""
    h0 = np.asarray(h0, np.float32)
    t = np.asarray(t, np.float32)
    W1 = np.asarray(W1, np.float32)
    b1 = np.asarray(b1, np.float32)
    W2 = np.asarray(W2, np.float32)
    b2 = np.asarray(b2, np.float32)
    W3 = np.asarray(W3, np.float32)
    b3 = np.asarray(b3, np.float32)

    dts = (t[1:] - t[:-1]).astype(np.float32)
    n = len(dts)
    dtm = np.float32(dts.mean())

    M = (W3.astype(np.float64) @ W1.astype(np.float64))
    Wp = np.linalg.pinv(W1.astype(np.float64))  # [HT, H]
    v = (b3.astype(np.float64) @ W1.astype(np.float64))  # [HT]
    cumf = np.concatenate([[0.0], np.cumsum(dts.astype(np.float64))])  # [n+1]
    cum = cumf[:n]  # C_i for steps

    bias0 = (b1[None, :].astype(np.float64) + cum[:, None] * v[None, :]).T
    bias12 = (b1[None, :].astype(np.float64)
              + (cum + 0.5 * dts.astype(np.float64))[:, None] * v[None, :]).T
    bias3 = (b1[None, :].astype(np.float64)
             + (cum + dts.astype(np.float64))[:, None] * v[None, :]).T
    # output correction: h_i = u_i @ Wp + C_i*b3, for i = 0..n
    tb3c = (cumf[:, None] * b3[None, :].astype(np.float64)).T  # [H, n+1]

    c = np.ascontiguousarray
    common = {
        "w1": c(W1),
        "w2": c(W2),
        "mc05": c((0.5 * dtm * M).astype(np.float32)),
        "mcd": c((dtm * M).astype(np.float32)),
        "msc": c(((dtm / 6.0) * M).astype(np.float32)),
        "msc2": c(((dtm / 3.0) * M).astype(np.float32)),
        "wp": c(Wp.astype(np.float32)),
        "iden": c(np.eye(HT, dtype=np.float32)),
        "b2c": c(b2.reshape(HT, 1)),
        "bias0": c(bias0.astype(np.float32)),
        "bias12": c(bias12.astype(np.float32)),
        "bias3": c(bias3.astype(np.float32)),
        "tb3c": c(tb3c.astype(np.float32)),
    }
    h0c = c(h0[core * b_local:(core + 1) * b_local].T)
    return {**common, "h0t": h0c}


def build_program(dts: np.ndarray, b_local: int, mm_fast: bool = True,
                  reps: int = 1, timing_mode: bool = False) -> bass.Bass:
    n_steps = len(dts)
    T = (OUT_GROUP + 1) if timing_mode else n_steps + 1
    cw = b_local // NSTREAM

    nc = bass.Bass(trn_type="TRN2", target_bir_lowering=False, debug=False)

    h0t = nc.dram_tensor("h0t", [H, b_local], F32, kind="ExternalInput").ap()
    w1 = nc.dram_tensor("w1", [H, HT], F32, kind="ExternalInput").ap()
    w2 = nc.dram_tensor("w2", [HT, HT], F32, kind="ExternalInput").ap()
    mc05 = nc.dram_tensor("mc05", [HT, HT], F32, kind="ExternalInput").ap()
    mcd = nc.dram_tensor("mcd", [HT, HT], F32, kind="ExternalInput").ap()
    msc = nc.dram_tensor("msc", [HT, HT], F32, kind="ExternalInput").ap()
    msc2 = nc.dram_tensor("msc2", [HT, HT], F32, kind="ExternalInput").ap()
    wp = nc.dram_tensor("wp", [HT, H], F32, kind="ExternalInput").ap()
    iden = nc.dram_tensor("iden", [HT, HT], F32, kind="ExternalInput").ap()
    b2d = nc.dram_tensor("b2c", [HT, 1], F32, kind="ExternalInput").ap()
    bias0 = nc.dram_tensor("bias0", [HT, n_steps], F32, kind="ExternalInput").ap()
    bias12 = nc.dram_tensor("bias12", [HT, n_steps], F32, kind="ExternalInput").ap()
    bias3 = nc.dram_tensor("bias3", [HT, n_steps], F32, kind="ExternalInput").ap()
    tb3c = nc.dram_tensor("tb3c", [H, n_steps + 1], F32, kind="ExternalInput").ap()
    out = nc.dram_tensor("out", [H, NSTREAM, T, cw], F32,
                         kind="ExternalOutput").ap()

    MMDT = F32R if mm_fast else F32

    with tile.TileContext(nc) as tc:
        with (
            tc.tile_pool(name="const", bufs=1) as cp,
            tc.tile_pool(name="sb", bufs=1) as sb,
            tc.tile_pool(name="ps", bufs=1, space="PSUM") as ps,
            tc.tile_pool(name="pu", bufs=1, space="PSUM") as pu,
        ):
            # Startup DMAs issue serially at ~650ns each on a queue's SEQ,
            # in EMISSION order - so emit the first-act critical path
            # (h0, W1, bias0) on SP first, and push the stationary staging
            # loads onto the DVE queue (idle until the converts anyway).
            W1f = cp.tile([H, HT], F32, tag="w1f")
            nc.sync.dma_start(out=W1f[:], in_=w1)
            h0_tiles = []
            for s in range(NSTREAM):
                c0 = s * cw
                h0s = sb.tile([H, cw], F32, tag=f"h0_{s}", name="h0s")
                nc.sync.dma_start(out=h0s[:], in_=h0t[:, c0:c0 + cw])
                h0_tiles.append(h0s)
            bias0t = cp.tile([HT, n_steps], F32, tag="bias0")
            nc.sync.dma_start(out=bias0t[:], in_=bias0)
            b2t = cp.tile([HT, 1], F32, tag="b2")
            nc.sync.dma_start(out=b2t[:], in_=b2d)

            # stationary weights: DMA fp32 staging (DVE queue) then DVE
            # round to f32r; w2/iden/mc05 first (needed earliest)
            stat = {}
            for nm, src, shp in (
                ("w2", w2, [HT, HT]), ("iden", iden, [HT, HT]),
                ("mc05", mc05, [HT, HT]), ("mcd", mcd, [HT, HT]),
                ("msc", msc, [HT, HT]), ("msc2", msc2, [HT, HT]),
                ("wp", wp, [HT, H]),
            ):
                dst = cp.tile(shp, MMDT, tag=nm)
                if mm_fast:
                    stg = sb.tile(shp, F32, tag=f"{nm}s", name="wstage")
                    nc.sync.dma_start(out=stg[:], in_=src)
                    nc.vector.tensor_copy(dst[:], stg[:])
                else:
                    nc.sync.dma_start(out=dst[:], in_=src)
                stat[nm] = dst

            bias12t = cp.tile([HT, n_steps], F32, tag="bias12")
            bias3t = cp.tile([HT, n_steps], F32, tag="bias3")
            tb3ct = cp.tile([H, n_steps + 1], F32, tag="tb3c")
            for dst, src in ((bias12t, bias12), (bias3t, bias3),
                             (tb3ct, tb3c)):
                nc.sync.dma_start(out=dst[:], in_=src)

            for _rep in range(reps):
                # --- init: u bank = W1.T @ h0 (per stream); t=0 output ---
                u_bank = []
                u_mm = [None] * NSTREAM
                stage_cur = [None] * NSTREAM
                for s in range(NSTREAM):
                    c0 = s * cw
                    if _rep == 0:
                        h0s = h0_tiles[s]
                    else:
                        h0s = sb.tile([H, cw], F32, tag=f"h0_{s}", name="h0s")
                        nc.sync.dma_start(out=h0s[:], in_=h0t[:, c0:c0 + cw])
                    nc.sync.dma_start(out=out[:, s, 0, :], in_=h0s[:])
                    ub = pu.tile([HT, cw], F32, tag=f"u_{s}", bufs=1, name="ub")
                    nc.tensor.matmul(ub[:], W1f[:], h0s[:], start=True,
                                     stop=True)
                    u_bank.append(ub)

                def out_row(s, i):
                    """Emit output row i: hout = Wp.T @ u_mm (PE, off-chain),
                    stage slot = hout + C_i*b3 (DVE tensor_scalar), and the
                    group DMA flush.  Call AFTER u_mm[s] for step i exists.
                    Everything here depends only on u_mm - no h recursion."""
                    ho = ps.tile([H, cw], F32, tag="hout", bufs=HOUT_BUFS,
                                 name="ho")
                    nc.tensor.matmul(ho[:], stat["wp"][:], u_mm[s][:],
                                     start=True, stop=True)
                    k = (i - 1) % OUT_GROUP
                    if k == 0:
                        stage_cur[s] = sb.tile([H, OUT_GROUP * cw], F32,
                                               tag=f"stage_{s}", bufs=2,
                                               name="stage")
                    stg = stage_cur[s]
                    hn = stg[:, k * cw:(k + 1) * cw]
                    nc.vector.tensor_scalar_add(hn, ho[:], tb3ct[:, i:i + 1])
                    if k == OUT_GROUP - 1 or i == n_steps:
                        src = stg[:, :(k + 1) * cw]
                        src = src.rearrange("h (t c) -> h t c", c=cw)
                        t0o = 1 if timing_mode else i - k
                        nc.sync.dma_start(
                            out=out[:, s, t0o:t0o + k + 1, :], in_=src)

                def stream_step(s, i):
                    """Emission granularity: (partA, partB) x 4 evals.  With
                    2 streams alternating segments, the Activation engine's
                    in-order queue becomes A.z1_e, B.z1_e, A.z2_e, B.z2_e,
                    ... so every act's upstream matmul (~175ns) completes
                    during the other stream's act (~400ns): Act runs
                    back-to-back at ~100%.  Premix start-matmuls (I@u) are
                    emitted where their deps are already satisfied so the
                    in-order PE queue never head-blocks on them."""
                    ub = u_bank[s]

                    P = [None] * 4  # premix banks for e=1..3
                    um = None

                    for e in range(4):
                        # partA: layer-1 tanh, layer-2 matmul
                        z1 = sb.tile([HT, cw], MMDT, tag=f"z1_{s}",
                                     bufs=Z1_BUFS, name="z1")
                        if e == 0:
                            nc.scalar.activation(z1[:], ub[:], AF.Tanh,
                                                 bias=bias0t[:, i:i + 1])
                        else:
                            bt = bias12t if e < 3 else bias3t
                            nc.scalar.activation(z1[:], P[e][:], AF.Tanh,
                                                 bias=bt[:, i:i + 1])
                        zp = ps.tile([HT, cw], F32, tag=f"zp_{s}",
                                     bufs=ZP_BUFS, name="zp")
                        nc.tensor.matmul(zp[:], stat["w2"][:], z1[:],
                                         start=True, stop=True)
                        if e == 0:
                            # u_mm copy AFTER the z1_0 act: Tile chains
                            # same-tile readers in emission order, so the
                            # act must come first or it inherits the copy's
                            # DVE latency at every step boundary.
                            um = sb.tile([HT, cw], MMDT, tag=f"umm_{s}",
                                         bufs=2, name="umm")
                            nc.vector.tensor_copy(um[:], ub[:])
                            u_mm[s] = um
                        yield

                        # partB: layer-2 tanh, then PE work ordered
                        # chain-critical first: premix stop (feeds next z1
                        # act) / final u accumulate (feeds next step's z1_0)
                        # before the off-chain RK4 accumulations.
                        z2 = sb.tile([HT, cw], MMDT, tag=f"z2_{s}",
                                     bufs=Z2_BUFS, name="z2")
                        nc.scalar.activation(z2[:], zp[:], AF.Tanh, bias=b2t[:])
                        mu = stat["msc"] if e in (0, 3) else stat["msc2"]
                        if e < 3:
                            # premix start here (deps long satisfied: z1_e
                            # act freed the single P bank, u_mm is ready) so
                            # PE executes it inside the z2-act stall window
                            # instead of head-blocking the other stream.
                            pb = ps.tile([HT, cw], F32, tag=f"P_{s}",
                                         bufs=P_BUFS, name="Pe")
                            nc.tensor.matmul(pb[:], stat["iden"][:], um[:],
                                             start=True, stop=False,
                                             skip_group_check=True)
                            P[e + 1] = pb
                            mc = stat["mc05"] if e < 2 else stat["mcd"]
                            nc.tensor.matmul(P[e + 1][:], mc[:], z2[:],
                                             start=False, stop=True,
                                             skip_group_check=True)
                        nc.tensor.matmul(ub[:], mu[:], z2[:], start=False,
                                         stop=(e == 3), skip_group_check=True)
                        if e == 0 and i > 0:
                            # output row i (reads u_mm of THIS step = u_i);
                            # placed here so the hout matmul sits behind the
                            # chain-critical premix work in the PE queue and
                            # its u_mm wait is satisfied by now.
                            out_row(s, i)
                        yield

                for i in range(n_steps):
                    gens = [stream_step(s, i) for s in range(NSTREAM)]
                    alive = list(gens)
                    while alive:
                        for g in list(alive):
                            try:
                                next(g)
                            except StopIteration:
                                alive.remove(g)
                # final row n_steps: u copy + reconstruction + flush
                for s in range(NSTREAM):
                    um = sb.tile([HT, cw], MMDT, tag=f"umm_{s}", bufs=2,
                                 name="umm")
                    nc.vector.tensor_copy(um[:], u_bank[s][:])
                    u_mm[s] = um
                    out_row(s, n_steps)
    return nc


def kernel(h0, t, W1, b1, W2, b2, W3, b3):
    h0 = np.ascontiguousarray(np.asarray(h0, dtype=np.float32))
    t = np.asarray(t, dtype=np.float32)

    B = h0.shape[0]
    T = t.shape[0]
    b_local = B // N_CORES

    dts = (t[1:] - t[:-1]).astype(np.float32)
    nc = build_program(dts, b_local, mm_fast=MM_FAST)
    _legalize_waits(nc)

    in_maps = [make_feeds(h0, t, W1, b1, W2, b2, W3, b3, b_local, c)
               for c in range(N_CORES)]
    res = run_bass_kernel_spmd(nc, in_maps, list(range(N_CORES)))
    global LAST_RESULTS
    LAST_RESULTS = res

    full = np.empty((B, T, h0.shape[1]), np.float32)
    for c in range(N_CORES):
        o = res.results[c]["out"]  # [H, NSTREAM, T, cw]
        full[c * b_local:(c + 1) * b_local] = (
            o.transpose(1, 3, 2, 0).reshape(b_local, T, h0.shape[1]))
    return full


LAST_RESULTS = None


# revision 3
# speedup vs baseline: 1.0007x; 1.0007x over previous
"""Trainium2 Bass kernel v2: RK4 neural-ODE solver, u-space formulation.

Reference: f(h) = tanh(tanh(h@W1+b1)@W2+b2)@W3 + b3, RK4 over 199 steps,
outputs all states [B, T, H].

Key reformulation (vs the v1 kernel)
------------------------------------
Track u := h@W1 - C(t)*(b3@W1) (layer-1 preactivation, drift-corrected)
instead of feeding h through W1 every eval.  Since layers 1 and 3 are
linear, with M := W3@W1 [100,100]:

  z1p_e   = u + c_e * M.T@z2_{e-1} + bias_e(t)      (c_1=c_2=dt/2, c_3=dt)
  u_next  = u + (dt/6) * M.T@(z2_0 + 2 z2_1 + 2 z2_2 + z2_3)
  h_i     = u_i @ pinv(W1) + C_i*b3                 (output reconstruction)

All b1/b3 contributions fold into per-step host-precomputed bias tables
(exact per-step dt); the c_e / dt/6 scales fold into pre-scaled stationary
weights (mean-dt; per-step deviation is ~1e-6 relative, far below tol).
W1 is full-rank 64x100 (cond ~11), so the pinv reconstruction is exact in
real arithmetic and adds only ~5e-4 relative noise in f32r - and output
errors do NOT feed back into the state.

Consequences:
  * Critical chain per eval is exactly act -> mm -> act -> mm (4 hops,
    2 engines); DVE has NO ops on the chain.
  * u lives in a persistent PSUM bank updated purely by PE accumulation
    (start=False matmuls with stationaries (dt/6)M, (dt/3)M) - exact f32.
  * The step boundary collapses to: act(z2_3) -> mm((dt/6)M @ z2_3 into u)
    -> act(z1_0 next step, reading the u bank directly with bias).
  * Premix banks P_e = I@u_mm (off-chain) + c_e M@z2_{e-1} (on chain)
    feed the e>=1 layer-1 activations; u_mm is a once-per-step DVE f32r
    copy of the u bank - which ALSO feeds the output matmul pinv(W1)@u_mm,
    so there is no h-state recursion at all outside PSUM.

Layout: batch 4096 -> 8 cores x 512; per core NSTREAM independent
streams pipeline against each other (per-half-eval interleave keeps the
in-order Activation queue dense).  Output written time-major, staged
OUT_GROUP steps per DMA, host-transposed.
"""

import os
import sys

import numpy as np

for _p in ("/opt/trn_rl_repo", "/root/.axon_site/_ro/trn_rl_repo"):
    if os.path.isdir(_p) and _p not in sys.path:
        sys.path.insert(0, _p)

os.environ.setdefault("NEURON_SCRATCHPAD_PAGE_SIZE", "4096")

import concourse.bass as bass
import concourse.mybir as mybir
import concourse.tile as tile
from concourse.bass_utils import run_bass_kernel_spmd

F32 = mybir.dt.float32
F32R = mybir.dt.float32r
AF = mybir.ActivationFunctionType
OP = mybir.AluOpType

N_CORES = 8
H = 64
HT = 100
NSTREAM = 2

OUT_GROUP = 8
Z1_BUFS = 2
Z2_BUFS = 3
# PSUM is 8 banks of 2KB/partition; tiles round up to a full bank. Per
# stream: u, P (premix), zp (layer-2) bufs=1 (every WAR edge they add is
# already implied by the act chain order) = 6 banks; hout (output recon)
# is a shared tag with bufs=2 = 2 banks.  Total 8.
P_BUFS = 1
ZP_BUFS = 1
HOUT_BUFS = 2
MM_FAST = True


def _legalize_waits(nc: bass.Bass, max_waits: int = 1) -> int:
    """Walrus encodes at most ONE sync-wait per instruction; hoist extras
    onto injected same-engine NoOps (engine streams are in-order)."""
    self_sem_prefix = {
        mybir.EngineType.Activation: "Activation_",
        mybir.EngineType.PE: "PE_",
        mybir.EngineType.DVE: "DVE_",
        mybir.EngineType.Pool: "Pool_",
    }
    n_new = 0
    for fn in nc.m.functions:
        for bb in fn.blocks:
            new_list = []
            changed = False
            for ins in bb.instructions:
                si = ins.sync_info
                waits = list(si.on_wait) if si and si.on_wait else []
                pref = self_sem_prefix.get(ins.engine)
                if pref is not None and any(
                    (w.ant_name or "").startswith(pref) for w in waits
                ):
                    waits = [w for w in waits
                             if not (w.ant_name or "").startswith(pref)]
                    ins.sync_info = mybir.SyncInfo(
                        on_wait=list(waits),
                        on_update=list(si.on_update) if si.on_update else [],
                    )
                    changed = True
                    si = ins.sync_info
                if len(waits) > max_waits:
                    keep = waits[-max_waits:]
                    for w in waits[:-max_waits]:
                        nop = mybir.InstNoOp(name=f"I-waitsplit-{n_new}")
                        n_new += 1
                        nop.engine = ins.engine
                        nop.sync_info = mybir.SyncInfo(on_wait=[w], on_update=[])
                        new_list.append(nop)
                    ins.sync_info = mybir.SyncInfo(
                        on_wait=keep,
                        on_update=list(si.on_update) if si.on_update else [],
                    )
                    changed = True
                new_list.append(ins)
            if changed:
                bb.instructions = new_list
    return n_new


def make_feeds(h0, t, W1, b1, W2, b2, W3, b3, b_local, core):
    """Host-side precompute: per-core input map for build_program's tensors."""
    h0 = np.asarray(h0, np.float32)
    t = np.asarray(t, np.float32)
    W1 = np.asarray(W1, np.float32)
    b1 = np.asarray(b1, np.float32)
    W2 = np.asarray(W2, np.float32)
    b2 = np.asarray(b2, np.float32)
    W3 = np.asarray(W3, np.float32)
    b3 = np.asarray(b3, np.float32)

    dts = (t[1:] - t[:-1]).astype(np.float32)
    n = len(dts)
    dtm = np.float32(dts.mean())

    M = (W3.astype(np.float64) @ W1.astype(np.float64))
    Wp = np.linalg.pinv(W1.astype(np.float64))  # [HT, H]
    v = (b3.astype(np.float64) @ W1.astype(np.float64))  # [HT]
    cumf = np.concatenate([[0.0], np.cumsum(dts.astype(np.float64))])  # [n+1]
    cum = cumf[:n]  # C_i for steps

    bias0 = (b1[None, :].astype(np.float64) + cum[:, None] * v[None, :]).T
    bias12 = (b1[None, :].astype(np.float64)
              + (cum + 0.5 * dts.astype(np.float64))[:, None] * v[None, :]).T
    bias3 = (b1[None, :].astype(np.float64)
             + (cum + dts.astype(np.float64))[:, None] * v[None, :]).T
    # output correction: h_i = u_i @ Wp + C_i*b3, for i = 0..n
    tb3c = (cumf[:, None] * b3[None, :].astype(np.float64)).T  # [H, n+1]

    c = np.ascontiguousarray
    common = {
        "w1": c(W1),
        "w2": c(W2),
        "mc05": c((0.5 * dtm * M).astype(np.float32)),
        "mcd": c((dtm * M).astype(np.float32)),
        "msc": c(((dtm / 6.0) * M).astype(np.float32)),
        "msc2": c(((dtm / 3.0) * M).astype(np.float32)),
        "wp": c(Wp.astype(np.float32)),
        "iden": c(np.eye(HT, dtype=np.float32)),
        "b2c": c(b2.reshape(HT, 1)),
        "bias0": c(bias0.astype(np.float32)),
        "bias12": c(bias12.astype(np.float32)),
        "bias3": c(bias3.astype(np.float32)),
        "tb3c": c(tb3c.astype(np.float32)),
    }
    h0c = c(h0[core * b_local:(core + 1) * b_local].T)
    return {**common, "h0t": h0c}


def build_program(dts: np.ndarray, b_local: int, mm_fast: bool = True,
                  reps: int = 1, timing_mode: bool = False) -> bass.Bass:
    n_steps = len(dts)
    T = (OUT_GROUP + 1) if timing_mode else n_steps + 1
    cw = b_local // NSTREAM

    nc = bass.Bass(trn_type="TRN2", target_bir_lowering=False, debug=False)

    h0t = nc.dram_tensor("h0t", [H, b_local], F32, kind="ExternalInput").ap()
    w1 = nc.dram_tensor("w1", [H, HT], F32, kind="ExternalInput").ap()
    w2 = nc.dram_tensor("w2", [HT, HT], F32, kind="ExternalInput").ap()
    mc05 = nc.dram_tensor("mc05", [HT, HT], F32, kind="ExternalInput").ap()
    mcd = nc.dram_tensor("mcd", [HT, HT], F32, kind="ExternalInput").ap()
    msc = nc.dram_tensor("msc", [HT, HT], F32, kind="ExternalInput").ap()
    msc2 = nc.dram_tensor("msc2", [HT, HT], F32, kind="ExternalInput").ap()
    wp = nc.dram_tensor("wp", [HT, H], F32, kind="ExternalInput").ap()
    iden = nc.dram_tensor("iden", [HT, HT], F32, kind="ExternalInput").ap()
    b2d = nc.dram_tensor("b2c", [HT, 1], F32, kind="ExternalInput").ap()
    bias0 = nc.dram_tensor("bias0", [HT, n_steps], F32, kind="ExternalInput").ap()
    bias12 = nc.dram_tensor("bias12", [HT, n_steps], F32, kind="ExternalInput").ap()
    bias3 = nc.dram_tensor("bias3", [HT, n_steps], F32, kind="ExternalInput").ap()
    tb3c = nc.dram_tensor("tb3c", [H, n_steps + 1], F32, kind="ExternalInput").ap()
    out = nc.dram_tensor("out", [H, NSTREAM, T, cw], F32,
                         kind="ExternalOutput").ap()

    MMDT = F32R if mm_fast else F32

    with tile.TileContext(nc) as tc:
        with (
            tc.tile_pool(name="const", bufs=1) as cp,
            tc.tile_pool(name="sb", bufs=1) as sb,
            tc.tile_pool(name="ps", bufs=1, space="PSUM") as ps,
            tc.tile_pool(name="pu", bufs=1, space="PSUM") as pu,
        ):
            # Startup DMAs issue serially at ~650ns each on a queue's SEQ,
            # in EMISSION order - so emit the first-act critical path
            # (h0, W1, bias0) on SP first, and push the stationary staging
            # loads onto the DVE queue (idle until the converts anyway).
            W1f = cp.tile([H, HT], F32, tag="w1f")
            nc.sync.dma_start(out=W1f[:], in_=w1)
            h0_tiles = []
            for s in range(NSTREAM):
                c0 = s * cw
                h0s = sb.tile([H, cw], F32, tag=f"h0_{s}", name="h0s")
                nc.sync.dma_start(out=h0s[:], in_=h0t[:, c0:c0 + cw])
                h0_tiles.append(h0s)
            bias0t = cp.tile([HT, n_steps], F32, tag="bias0")
            nc.sync.dma_start(out=bias0t[:], in_=bias0)
            b2t = cp.tile([HT, 1], F32, tag="b2")
            nc.sync.dma_start(out=b2t[:], in_=b2d)

            # stationary weights (DMA fp32 staging then DVE round to f32r)
            # and remaining bias tables, all emitted in FIRST-USE order:
            # issue slots are ~650ns apiece on SP.SEQ, so late emission of
            # an early-needed table stalls the first step.
            stat = {}
            bias12t = cp.tile([HT, n_steps], F32, tag="bias12")
            bias3t = cp.tile([HT, n_steps], F32, tag="bias3")
            tb3ct = cp.tile([H, n_steps + 1], F32, tag="tb3c")
            tables = {"bias12": (bias12t, bias12), "bias3": (bias3t, bias3),
                      "tb3c": (tb3ct, tb3c)}
            for nm, src, shp in (
                ("w2", w2, [HT, HT]), ("iden", iden, [HT, HT]),
                ("mc05", mc05, [HT, HT]), ("bias12", None, None),
                ("msc", msc, [HT, HT]), ("mcd", mcd, [HT, HT]),
                ("bias3", None, None), ("msc2", msc2, [HT, HT]),
                ("wp", wp, [HT, H]), ("tb3c", None, None),
            ):
                if nm in tables:
                    dst, tsrc = tables[nm]
                    nc.sync.dma_start(out=dst[:], in_=tsrc)
                    continue
                dst = cp.tile(shp, MMDT, tag=nm)
                if mm_fast:
                    stg = sb.tile(shp, F32, tag=f"{nm}s", name="wstage")
                    nc.sync.dma_start(out=stg[:], in_=src)
                    nc.vector.tensor_copy(dst[:], stg[:])
                else:
                    nc.sync.dma_start(out=dst[:], in_=src)
                stat[nm] = dst

            for _rep in range(reps):
                # --- init: u bank = W1.T @ h0 (per stream); t=0 output ---
                u_bank = []
                u_mm = [None] * NSTREAM
                stage_cur = [None] * NSTREAM
                for s in range(NSTREAM):
                    c0 = s * cw
                    if _rep == 0:
                        h0s = h0_tiles[s]
                    else:
                        h0s = sb.tile([H, cw], F32, tag=f"h0_{s}", name="h0s")
                        nc.sync.dma_start(out=h0s[:], in_=h0t[:, c0:c0 + cw])
                    nc.sync.dma_start(out=out[:, s, 0, :], in_=h0s[:])
                    ub = pu.tile([HT, cw], F32, tag=f"u_{s}", bufs=1, name="ub")
                    nc.tensor.matmul(ub[:], W1f[:], h0s[:], start=True,
                                     stop=True)
                    u_bank.append(ub)

                def out_row(s, i):
                    """Emit output row i: hout = Wp.T @ u_mm (PE, off-chain),
                    stage slot = hout + C_i*b3 (DVE tensor_scalar), and the
                    group DMA flush.  Call AFTER u_mm[s] for step i exists.
                    Everything here depends only on u_mm - no h recursion."""
                    ho = ps.tile([H, cw], F32, tag="hout", bufs=HOUT_BUFS,
                                 name="ho")
                    nc.tensor.matmul(ho[:], stat["wp"][:], u_mm[s][:],
                                     start=True, stop=True)
                    k = (i - 1) % OUT_GROUP
                    if k == 0:
                        stage_cur[s] = sb.tile([H, OUT_GROUP * cw], F32,
                                               tag=f"stage_{s}", bufs=2,
                                               name="stage")
                    stg = stage_cur[s]
                    hn = stg[:, k * cw:(k + 1) * cw]
                    nc.vector.tensor_scalar_add(hn, ho[:], tb3ct[:, i:i + 1])
                    if k == OUT_GROUP - 1 or i == n_steps:
                        src = stg[:, :(k + 1) * cw]
                        src = src.rearrange("h (t c) -> h t c", c=cw)
                        t0o = 1 if timing_mode else i - k
                        nc.sync.dma_start(
                            out=out[:, s, t0o:t0o + k + 1, :], in_=src)

                def stream_step(s, i):
                    """Emission granularity: (partA, partB) x 4 evals.  With
                    2 streams alternating segments, the Activation engine's
                    in-order queue becomes A.z1_e, B.z1_e, A.z2_e, B.z2_e,
                    ... so every act's upstream matmul (~175ns) completes
                    during the other stream's act (~400ns): Act runs
                    back-to-back at ~100%.  Premix start-matmuls (I@u) are
                    emitted where their deps are already satisfied so the
                    in-order PE queue never head-blocks on them."""
                    ub = u_bank[s]

                    P = [None] * 4  # premix banks for e=1..3
                    um = None

                    for e in range(4):
                        # partA: layer-1 tanh, layer-2 matmul
                        z1 = sb.tile([HT, cw], MMDT, tag=f"z1_{s}",
                                     bufs=Z1_BUFS, name="z1")
                        if e == 0:
                            nc.scalar.activation(z1[:], ub[:], AF.Tanh,
                                                 bias=bias0t[:, i:i + 1])
                        else:
                            bt = bias12t if e < 3 else bias3t
                            nc.scalar.activation(z1[:], P[e][:], AF.Tanh,
                                                 bias=bt[:, i:i + 1])
                        zp = ps.tile([HT, cw], F32, tag=f"zp_{s}",
                                     bufs=ZP_BUFS, name="zp")
                        nc.tensor.matmul(zp[:], stat["w2"][:], z1[:],
                                         start=True, stop=True)
                        if e == 0:
                            # u_mm copy AFTER the z1_0 act: Tile chains
                            # same-tile readers in emission order, so the
                            # act must come first or it inherits the copy's
                            # DVE latency at every step boundary.
                            um = sb.tile([HT, cw], MMDT, tag=f"umm_{s}",
                                         bufs=2, name="umm")
                            nc.vector.tensor_copy(um[:], ub[:])
                            u_mm[s] = um
                        yield

                        # partB: layer-2 tanh, then PE work ordered
                        # chain-critical first: premix stop (feeds next z1
                        # act) / final u accumulate (feeds next step's z1_0)
                        # before the off-chain RK4 accumulations.
                        z2 = sb.tile([HT, cw], MMDT, tag=f"z2_{s}",
                                     bufs=Z2_BUFS, name="z2")
                        nc.scalar.activation(z2[:], zp[:], AF.Tanh, bias=b2t[:])
                        mu = stat["msc"] if e in (0, 3) else stat["msc2"]
                        if e < 3:
                            # premix start here (deps long satisfied: z1_e
                            # act freed the single P bank, u_mm is ready) so
                            # PE executes it inside the z2-act stall window
                            # instead of head-blocking the other stream.
                            pb = ps.tile([HT, cw], F32, tag=f"P_{s}",
                                         bufs=P_BUFS, name="Pe")
                            nc.tensor.matmul(pb[:], stat["iden"][:], um[:],
                                             start=True, stop=False,
                                             skip_group_check=True)
                            P[e + 1] = pb
                            mc = stat["mc05"] if e < 2 else stat["mcd"]
                            nc.tensor.matmul(P[e + 1][:], mc[:], z2[:],
                                             start=False, stop=True,
                                             skip_group_check=True)
                        nc.tensor.matmul(ub[:], mu[:], z2[:], start=False,
                                         stop=(e == 3), skip_group_check=True)
                        if e == 0 and i > 0:
                            # output row i (reads u_mm of THIS step = u_i);
                            # placed here so the hout matmul sits behind the
                            # chain-critical premix work in the PE queue and
                            # its u_mm wait is satisfied by now.
                            out_row(s, i)
                        yield

                for i in range(n_steps):
                    gens = [stream_step(s, i) for s in range(NSTREAM)]
                    alive = list(gens)
                    while alive:
                        for g in list(alive):
                            try:
                                next(g)
                            except StopIteration:
                                alive.remove(g)
                # final row n_steps: u copy + reconstruction + flush
                for s in range(NSTREAM):
                    um = sb.tile([HT, cw], MMDT, tag=f"umm_{s}", bufs=2,
                                 name="umm")
                    nc.vector.tensor_copy(um[:], u_bank[s][:])
                    u_mm[s] = um
                    out_row(s, n_steps)
    return nc


def kernel(h0, t, W1, b1, W2, b2, W3, b3):
    h0 = np.ascontiguousarray(np.asarray(h0, dtype=np.float32))
    t = np.asarray(t, dtype=np.float32)

    B = h0.shape[0]
    T = t.shape[0]
    b_local = B // N_CORES

    dts = (t[1:] - t[:-1]).astype(np.float32)
    nc = build_program(dts, b_local, mm_fast=MM_FAST)
    _legalize_waits(nc)

    in_maps = [make_feeds(h0, t, W1, b1, W2, b2, W3, b3, b_local, c)
               for c in range(N_CORES)]
    res = run_bass_kernel_spmd(nc, in_maps, list(range(N_CORES)))
    global LAST_RESULTS
    LAST_RESULTS = res

    full = np.empty((B, T, h0.shape[1]), np.float32)
    for c in range(N_CORES):
        o = res.results[c]["out"]  # [H, NSTREAM, T, cw]
        full[c * b_local:(c + 1) * b_local] = (
            o.transpose(1, 3, 2, 0).reshape(b_local, T, h0.shape[1]))
    return full


LAST_RESULTS = None


# revision 4
# speedup vs baseline: 1.0020x; 1.0013x over previous
"""Trainium2 Bass kernel v2: RK4 neural-ODE solver, u-space formulation.

Reference: f(h) = tanh(tanh(h@W1+b1)@W2+b2)@W3 + b3, RK4 over 199 steps,
outputs all states [B, T, H].

Key reformulation (vs the v1 kernel)
------------------------------------
Track u := h@W1 - C(t)*(b3@W1) (layer-1 preactivation, drift-corrected)
instead of feeding h through W1 every eval.  Since layers 1 and 3 are
linear, with M := W3@W1 [100,100]:

  z1p_e   = u + c_e * M.T@z2_{e-1} + bias_e(t)      (c_1=c_2=dt/2, c_3=dt)
  u_next  = u + (dt/6) * M.T@(z2_0 + 2 z2_1 + 2 z2_2 + z2_3)
  h_i     = u_i @ pinv(W1) + C_i*b3                 (output reconstruction)

All b1/b3 contributions fold into per-step host-precomputed bias tables
(exact per-step dt); the c_e / dt/6 scales fold into pre-scaled stationary
weights (mean-dt; per-step deviation is ~1e-6 relative, far below tol).
W1 is full-rank 64x100 (cond ~11), so the pinv reconstruction is exact in
real arithmetic and adds only ~5e-4 relative noise in f32r - and output
errors do NOT feed back into the state.

Consequences:
  * Critical chain per eval is exactly act -> mm -> act -> mm (4 hops,
    2 engines); DVE has NO ops on the chain.
  * u lives in a persistent PSUM bank updated purely by PE accumulation
    (start=False matmuls with stationaries (dt/6)M, (dt/3)M) - exact f32.
  * The step boundary collapses to: act(z2_3) -> mm((dt/6)M @ z2_3 into u)
    -> act(z1_0 next step, reading the u bank directly with bias).
  * Premix banks P_e = I@u_mm (off-chain) + c_e M@z2_{e-1} (on chain)
    feed the e>=1 layer-1 activations; u_mm is a once-per-step DVE f32r
    copy of the u bank - which ALSO feeds the output matmul pinv(W1)@u_mm,
    so there is no h-state recursion at all outside PSUM.

Layout: batch 4096 -> 8 cores x 512; per core NSTREAM independent
streams pipeline against each other (per-half-eval interleave keeps the
in-order Activation queue dense).  Output written time-major, staged
OUT_GROUP steps per DMA, host-transposed.
"""

import os
import sys

import numpy as np

for _p in ("/opt/trn_rl_repo", "/root/.axon_site/_ro/trn_rl_repo"):
    if os.path.isdir(_p) and _p not in sys.path:
        sys.path.insert(0, _p)

os.environ.setdefault("NEURON_SCRATCHPAD_PAGE_SIZE", "4096")

import concourse.bass as bass
import concourse.mybir as mybir
import concourse.tile as tile
from concourse.bass_utils import run_bass_kernel_spmd

F32 = mybir.dt.float32
F32R = mybir.dt.float32r
AF = mybir.ActivationFunctionType
OP = mybir.AluOpType

N_CORES = 8
H = 64
HT = 100
NSTREAM = 2

OUT_GROUP = 8
Z1_BUFS = 2
Z2_BUFS = 3
# PSUM is 8 banks of 2KB/partition; tiles round up to a full bank. Per
# stream: u, P (premix), zp (layer-2) bufs=1 (every WAR edge they add is
# already implied by the act chain order) = 6 banks; hout (output recon)
# is a shared tag with bufs=2 = 2 banks.  Total 8.
P_BUFS = 1
ZP_BUFS = 1
HOUT_BUFS = 2
MM_FAST = True


def _legalize_waits(nc: bass.Bass, max_waits: int = 1) -> int:
    """Walrus encodes at most ONE sync-wait per instruction; hoist extras
    onto injected same-engine NoOps (engine streams are in-order)."""
    self_sem_prefix = {
        mybir.EngineType.Activation: "Activation_",
        mybir.EngineType.PE: "PE_",
        mybir.EngineType.DVE: "DVE_",
        mybir.EngineType.Pool: "Pool_",
    }
    n_new = 0
    for fn in nc.m.functions:
        for bb in fn.blocks:
            new_list = []
            changed = False
            for ins in bb.instructions:
                si = ins.sync_info
                waits = list(si.on_wait) if si and si.on_wait else []
                pref = self_sem_prefix.get(ins.engine)
                if pref is not None and any(
                    (w.ant_name or "").startswith(pref) for w in waits
                ):
                    waits = [w for w in waits
                             if not (w.ant_name or "").startswith(pref)]
                    ins.sync_info = mybir.SyncInfo(
                        on_wait=list(waits),
                        on_update=list(si.on_update) if si.on_update else [],
                    )
                    changed = True
                    si = ins.sync_info
                if len(waits) > max_waits:
                    keep = waits[-max_waits:]
                    for w in waits[:-max_waits]:
                        nop = mybir.InstNoOp(name=f"I-waitsplit-{n_new}")
                        n_new += 1
                        nop.engine = ins.engine
                        nop.sync_info = mybir.SyncInfo(on_wait=[w], on_update=[])
                        new_list.append(nop)
                    ins.sync_info = mybir.SyncInfo(
                        on_wait=keep,
                        on_update=list(si.on_update) if si.on_update else [],
                    )
                    changed = True
                new_list.append(ins)
            if changed:
                bb.instructions = new_list
    return n_new


def make_feeds(h0, t, W1, b1, W2, b2, W3, b3, b_local, core):
    """Host-side precompute: per-core input map for build_program's tensors."""
    h0 = np.asarray(h0, np.float32)
    t = np.asarray(t, np.float32)
    W1 = np.asarray(W1, np.float32)
    b1 = np.asarray(b1, np.float32)
    W2 = np.asarray(W2, np.float32)
    b2 = np.asarray(b2, np.float32)
    W3 = np.asarray(W3, np.float32)
    b3 = np.asarray(b3, np.float32)

    dts = (t[1:] - t[:-1]).astype(np.float32)
    n = len(dts)
    dtm = np.float32(dts.mean())

    M = (W3.astype(np.float64) @ W1.astype(np.float64))
    Wp = np.linalg.pinv(W1.astype(np.float64))  # [HT, H]
    v = (b3.astype(np.float64) @ W1.astype(np.float64))  # [HT]
    cumf = np.concatenate([[0.0], np.cumsum(dts.astype(np.float64))])  # [n+1]
    cum = cumf[:n]  # C_i for steps

    bias0 = (b1[None, :].astype(np.float64) + cum[:, None] * v[None, :]).T
    bias12 = (b1[None, :].astype(np.float64)
              + (cum + 0.5 * dts.astype(np.float64))[:, None] * v[None, :]).T
    bias3 = (b1[None, :].astype(np.float64)
             + (cum + dts.astype(np.float64))[:, None] * v[None, :]).T
    # output correction: h_i = u_i @ Wp + C_i*b3, for i = 0..n
    tb3c = (cumf[:, None] * b3[None, :].astype(np.float64)).T  # [H, n+1]

    c = np.ascontiguousarray
    # Everything except W1/h0/bias0 rides ONE packed tensor -> one DMA
    # issue slot instead of eleven (SP.SEQ issues serialize at ~650ns).
    # Layout (columns): w2@0 iden@100 mc05@200 mcd@300 msc@400 msc2@500
    # wp@600 b2@664 bias12@665 bias3@665+n tb3c@665+2n (rows 0:H)
    n = len(dts)
    pack1 = np.zeros((HT, 300), np.float32)  # earliest-needed stationaries
    pack1[:, 0:100] = W2
    pack1[:, 100:200] = np.eye(HT)
    pack1[:, 200:300] = (0.5 * dtm * M)
    pack2 = np.zeros((HT, 365 + 2 * n + (n + 1)), np.float32)
    pack2[:, 0:100] = (dtm * M)
    pack2[:, 100:200] = ((dtm / 6.0) * M)
    pack2[:, 200:300] = ((dtm / 3.0) * M)
    pack2[:, 300:364] = Wp
    pack2[:, 364] = b2
    pack2[:, 365:365 + n] = bias12
    pack2[:, 365 + n:365 + 2 * n] = bias3
    pack2[0:H, 365 + 2 * n:] = tb3c
    common = {
        "w1": c(W1),
        "bias0": c(bias0.astype(np.float32)),
        "wpack1": c(pack1),
        "wpack2": c(pack2),
    }
    h0c = c(h0[core * b_local:(core + 1) * b_local].T)
    return {**common, "h0t": h0c}


def build_program(dts: np.ndarray, b_local: int, mm_fast: bool = True,
                  reps: int = 1, timing_mode: bool = False) -> bass.Bass:
    n_steps = len(dts)
    T = (OUT_GROUP + 1) if timing_mode else n_steps + 1
    cw = b_local // NSTREAM

    nc = bass.Bass(trn_type="TRN2", target_bir_lowering=False, debug=False)

    h0t = nc.dram_tensor("h0t", [H, b_local], F32, kind="ExternalInput").ap()
    w1 = nc.dram_tensor("w1", [H, HT], F32, kind="ExternalInput").ap()
    bias0 = nc.dram_tensor("bias0", [HT, n_steps], F32, kind="ExternalInput").ap()
    PACKW = 365 + 2 * n_steps + (n_steps + 1)
    wpack1 = nc.dram_tensor("wpack1", [HT, 300], F32, kind="ExternalInput").ap()
    wpack2 = nc.dram_tensor("wpack2", [HT, PACKW], F32, kind="ExternalInput").ap()
    out = nc.dram_tensor("out", [H, NSTREAM, T, cw], F32,
                         kind="ExternalOutput").ap()

    MMDT = F32R if mm_fast else F32

    with tile.TileContext(nc) as tc:
        with (
            tc.tile_pool(name="const", bufs=1) as cp,
            tc.tile_pool(name="sb", bufs=1) as sb,
            tc.tile_pool(name="ps", bufs=1, space="PSUM") as ps,
            tc.tile_pool(name="pu", bufs=1, space="PSUM") as pu,
        ):
            # Startup DMAs issue serially at ~650ns each on SP.SEQ in
            # EMISSION order, and the serial issue rate (650) trails the
            # steady act cadence (425) - so the first-act critical path
            # (W1, h0, bias0) goes first, and EVERYTHING else rides one
            # packed tensor = one issue slot.  fp32 tables are then used
            # directly as slices of the packed tile; f32r stationaries are
            # DVE-converted out of it in first-use order.
            W1f = cp.tile([H, HT], F32, tag="w1f")
            nc.sync.dma_start(out=W1f[:], in_=w1)
            h0_tiles = []
            for s in range(NSTREAM):
                c0 = s * cw
                h0s = sb.tile([H, cw], F32, tag=f"h0_{s}", name="h0s")
                nc.sync.dma_start(out=h0s[:], in_=h0t[:, c0:c0 + cw])
                h0_tiles.append(h0s)
            bias0t = cp.tile([HT, n_steps], F32, tag="bias0")
            nc.sync.dma_start(out=bias0t[:], in_=bias0)
            pk1 = cp.tile([HT, 300], F32, tag="wpk1")
            nc.sync.dma_start(out=pk1[:], in_=wpack1)
            pk = cp.tile([HT, PACKW], F32, tag="wpk")
            nc.sync.dma_start(out=pk[:], in_=wpack2)

            stat = {}
            for nm, src_t, off, w_ in (
                ("w2", pk1, 0, HT), ("iden", pk1, 100, HT),
                ("mc05", pk1, 200, HT), ("msc", pk, 100, HT),
                ("mcd", pk, 0, HT), ("msc2", pk, 200, HT),
                ("wp", pk, 300, H),
            ):
                dst = cp.tile([HT, w_], MMDT, tag=nm)
                nc.vector.tensor_copy(dst[:], src_t[:, off:off + w_])
                stat[nm] = dst
            b2t = pk[:, 364:365]
            bias12t = pk[:, 365:365 + n_steps]
            bias3t = pk[:, 365 + n_steps:365 + 2 * n_steps]
            tb3ct = pk[0:H, 365 + 2 * n_steps:PACKW]

            for _rep in range(reps):
                # --- init: u bank = W1.T @ h0 (per stream); t=0 output ---
                u_bank = []
                u_mm = [None] * NSTREAM
                stage_cur = [None] * NSTREAM
                for s in range(NSTREAM):
                    c0 = s * cw
                    if _rep == 0:
                        h0s = h0_tiles[s]
                    else:
                        h0s = sb.tile([H, cw], F32, tag=f"h0_{s}", name="h0s")
                        nc.sync.dma_start(out=h0s[:], in_=h0t[:, c0:c0 + cw])
                    nc.sync.dma_start(out=out[:, s, 0, :], in_=h0s[:])
                    ub = pu.tile([HT, cw], F32, tag=f"u_{s}", bufs=1, name="ub")
                    nc.tensor.matmul(ub[:], W1f[:], h0s[:], start=True,
                                     stop=True)
                    u_bank.append(ub)

                def out_row(s, i):
                    """Emit output row i: hout = Wp.T @ u_mm (PE, off-chain),
                    stage slot = hout + C_i*b3 (DVE tensor_scalar), and the
                    group DMA flush.  Call AFTER u_mm[s] for step i exists.
                    Everything here depends only on u_mm - no h recursion."""
                    ho = ps.tile([H, cw], F32, tag="hout", bufs=HOUT_BUFS,
                                 name="ho")
                    nc.tensor.matmul(ho[:], stat["wp"][:], u_mm[s][:],
                                     start=True, stop=True)
                    k = (i - 1) % OUT_GROUP
                    if k == 0:
                        stage_cur[s] = sb.tile([H, OUT_GROUP * cw], F32,
                                               tag=f"stage_{s}", bufs=2,
                                               name="stage")
                    stg = stage_cur[s]
                    hn = stg[:, k * cw:(k + 1) * cw]
                    nc.vector.tensor_scalar_add(hn, ho[:], tb3ct[:, i:i + 1])
                    if k == OUT_GROUP - 1 or i == n_steps:
                        src = stg[:, :(k + 1) * cw]
                        src = src.rearrange("h (t c) -> h t c", c=cw)
                        t0o = 1 if timing_mode else i - k
                        nc.sync.dma_start(
                            out=out[:, s, t0o:t0o + k + 1, :], in_=src)

                def stream_step(s, i):
                    """Emission granularity: (partA, partB) x 4 evals.  With
                    2 streams alternating segments, the Activation engine's
                    in-order queue becomes A.z1_e, B.z1_e, A.z2_e, B.z2_e,
                    ... so every act's upstream matmul (~175ns) completes
                    during the other stream's act (~400ns): Act runs
                    back-to-back at ~100%.  Premix start-matmuls (I@u) are
                    emitted where their deps are already satisfied so the
                    in-order PE queue never head-blocks on them."""
                    ub = u_bank[s]

                    P = [None] * 4  # premix banks for e=1..3
                    um = None

                    for e in range(4):
                        # partA: layer-1 tanh, layer-2 matmul
                        z1 = sb.tile([HT, cw], MMDT, tag=f"z1_{s}",
                                     bufs=Z1_BUFS, name="z1")
                        if e == 0:
                            nc.scalar.activation(z1[:], ub[:], AF.Tanh,
                                                 bias=bias0t[:, i:i + 1])
                        else:
                            bt = bias12t if e < 3 else bias3t
                            nc.scalar.activation(z1[:], P[e][:], AF.Tanh,
                                                 bias=bt[:, i:i + 1])
                        zp = ps.tile([HT, cw], F32, tag=f"zp_{s}",
                                     bufs=ZP_BUFS, name="zp")
                        nc.tensor.matmul(zp[:], stat["w2"][:], z1[:],
                                         start=True, stop=True)
                        if e == 0:
                            # u_mm copy AFTER the z1_0 act: Tile chains
                            # same-tile readers in emission order, so the
                            # act must come first or it inherits the copy's
                            # DVE latency at every step boundary.
                            um = sb.tile([HT, cw], MMDT, tag=f"umm_{s}",
                                         bufs=2, name="umm")
                            nc.vector.tensor_copy(um[:], ub[:])
                            u_mm[s] = um
                        yield

                        # partB: layer-2 tanh, then PE work ordered
                        # chain-critical first: premix stop (feeds next z1
                        # act) / final u accumulate (feeds next step's z1_0)
                        # before the off-chain RK4 accumulations.
                        z2 = sb.tile([HT, cw], MMDT, tag=f"z2_{s}",
                                     bufs=Z2_BUFS, name="z2")
                        nc.scalar.activation(z2[:], zp[:], AF.Tanh, bias=b2t[:])
                        mu = stat["msc"] if e in (0, 3) else stat["msc2"]
                        if e < 3:
                            # premix start here (deps long satisfied: z1_e
                            # act freed the single P bank, u_mm is ready) so
                            # PE executes it inside the z2-act stall window
                            # instead of head-blocking the other stream.
                            pb = ps.tile([HT, cw], F32, tag=f"P_{s}",
                                         bufs=P_BUFS, name="Pe")
                            nc.tensor.matmul(pb[:], stat["iden"][:], um[:],
                                             start=True, stop=False,
                                             skip_group_check=True)
                            P[e + 1] = pb
                            mc = stat["mc05"] if e < 2 else stat["mcd"]
                            nc.tensor.matmul(P[e + 1][:], mc[:], z2[:],
                                             start=False, stop=True,
                                             skip_group_check=True)
                        nc.tensor.matmul(ub[:], mu[:], z2[:], start=False,
                                         stop=(e == 3), skip_group_check=True)
                        if e == 0 and i > 0:
                            # output row i (reads u_mm of THIS step = u_i);
                            # placed here so the hout matmul sits behind the
                            # chain-critical premix work in the PE queue and
                            # its u_mm wait is satisfied by now.
                            out_row(s, i)
                        yield

                for i in range(n_steps):
                    gens = [stream_step(s, i) for s in range(NSTREAM)]
                    alive = list(gens)
                    while alive:
                        for g in list(alive):
                            try:
                                next(g)
                            except StopIteration:
                                alive.remove(g)
                # final row n_steps: u copy + reconstruction + flush
                for s in range(NSTREAM):
                    um = sb.tile([HT, cw], MMDT, tag=f"umm_{s}", bufs=2,
                                 name="umm")
                    nc.vector.tensor_copy(um[:], u_bank[s][:])
                    u_mm[s] = um
                    out_row(s, n_steps)
    return nc


def kernel(h0, t, W1, b1, W2, b2, W3, b3):
    h0 = np.ascontiguousarray(np.asarray(h0, dtype=np.float32))
    t = np.asarray(t, dtype=np.float32)

    B = h0.shape[0]
    T = t.shape[0]
    b_local = B // N_CORES

    dts = (t[1:] - t[:-1]).astype(np.float32)
    nc = build_program(dts, b_local, mm_fast=MM_FAST)
    _legalize_waits(nc)

    in_maps = [make_feeds(h0, t, W1, b1, W2, b2, W3, b3, b_local, c)
               for c in range(N_CORES)]
    res = run_bass_kernel_spmd(nc, in_maps, list(range(N_CORES)))
    global LAST_RESULTS
    LAST_RESULTS = res

    full = np.empty((B, T, h0.shape[1]), np.float32)
    for c in range(N_CORES):
        o = res.results[c]["out"]  # [H, NSTREAM, T, cw]
        full[c * b_local:(c + 1) * b_local] = (
            o.transpose(1, 3, 2, 0).reshape(b_local, T, h0.shape[1]))
    return full


LAST_RESULTS = None
